# revision 1
# baseline (speedup 1.0000x reference)
"""Causal cross-attention (B=4, L=2048, D=1024, H=16, hd=64) on 8 trn2 cores.

Sharding: core c -> (batch b = c//2, head-group g = c%2 of 8 heads).
Each core computes QKV projections for its head group, causal-masked
per-head attention, and a partial output projection (its heads' columns
of Wo). Host sums the two partials per batch and adds bo.

On-chip layout is feature-major ("transposed") end to end so no on-chip
transposes are needed:
  QhT/KhT: [feat, seq], V: [seq, feat(+ones col)] -> scoresT = KhT_blk.T @ QhT_blk
  attn_outT accumulated as V_ext.T @ probsT with a ones column giving the
  softmax denominator for free; normalization via reciprocal + K=1
  broadcast matmul. All matmuls run in float32r (full PE rate at N>=512).
"""

import numpy as np
import ml_dtypes

B, L, D, H, HD = 4, 2048, 1024, 16, 64
NCORES = 8
SCALE = HD ** -0.5

_CACHE = {}


def _build_nc():
    import concourse.mybir as mybir
    import concourse.tile as tile
    from concourse import bacc

    F32 = mybir.dt.float32
    F32R = mybir.dt.float32r
    BF16 = mybir.dt.bfloat16
    AF = mybir.ActivationFunctionType
    ALU = mybir.AluOpType

    nc = bacc.Bacc("TRN2", target_bir_lowering=False, debug=False)

    qt_d = nc.declare_dram_parameter("qt", [4, 128, 4096], F32R, isOutput=False)
    kvt_d = nc.declare_dram_parameter("kvt", [4, 128, 4096], F32R, isOutput=False)
    wq_d = nc.declare_dram_parameter("wq", [128, 4096], F32R, isOutput=False)
    wk_d = nc.declare_dram_parameter("wk", [128, 4096], F32R, isOutput=False)
    wv_d = nc.declare_dram_parameter("wv", [128, 4096], F32R, isOutput=False)
    wo_d = nc.declare_dram_parameter("wo", [128, 4096], F32R, isOutput=False)
    bq_d = nc.declare_dram_parameter("bq", [128, 4], F32, isOutput=False)
    bk_d = nc.declare_dram_parameter("bk", [128, 4], F32, isOutput=False)
    bv_d = nc.declare_dram_parameter("bv", [1, 512], F32R, isOutput=False)
    ones_d = nc.declare_dram_parameter("ones", [1, 128], F32R, isOutput=False)
    ones8_d = nc.declare_dram_parameter("ones8", [128, 8], F32R, isOutput=False)
    msk_d = nc.declare_dram_parameter("msk", [128, 2048], BF16, isOutput=False)
    e2_d = nc.declare_dram_parameter("e2", [128, 128], F32R, isOutput=False)
    out_d = nc.declare_dram_parameter("out", [2048, 1024], F32, isOutput=True)

    with tile.TileContext(nc) as tc:
        with (
            tc.tile_pool(name="const", bufs=1) as const,
            tc.tile_pool(name="w", bufs=2) as wp,
            tc.tile_pool(name="stream", bufs=2) as stream,
            tc.tile_pool(name="khp", bufs=1) as khp,
            tc.tile_pool(name="vxp", bufs=1) as vxp,
            tc.tile_pool(name="qhp", bufs=2) as qhp,
            tc.tile_pool(name="atp", bufs=2) as atp,
            tc.tile_pool(name="prp", bufs=2) as prp,
            tc.tile_pool(name="smp", bufs=2) as smp,
            tc.tile_pool(name="osp", bufs=2) as osp,
            tc.tile_pool(name="ps_sc", bufs=1, space="PSUM") as ps_sc,
            tc.tile_pool(name="ps_out", bufs=1, space="PSUM") as ps_out,
            tc.tile_pool(name="ps_misc", bufs=2, space="PSUM") as ps_misc,
        ):
            def misc_ps(i, shape=(128, 512)):
                return ps_misc.tile(list(shape), F32, tag="misc", name="miscp")

            # tiny const loads go first: the bv-broadcast matmul is the PE's
            # first instruction, and the in-order PE would otherwise sit
            # blocked behind megabytes of weight DMAs waiting for these bytes
            bq_t = const.tile([128, 4], F32, tag="bq")
            bk_t = const.tile([128, 4], F32, tag="bk")
            ones_t = const.tile([1, 128], F32R, tag="ones")
            ones8_t = const.tile([128, 8], F32R, tag="ones8")
            bv_t = const.tile([1, 512], F32R, tag="bv")
            nc.sync.dma_start(bk_t[:], bk_d[:])
            nc.sync.dma_start(ones_t[:], ones_d[:])
            nc.sync.dma_start(ones8_t[:], ones8_d[:])
            nc.sync.dma_start(bv_t[:], bv_d[:])
            nc.sync.dma_start(bq_t[:], bq_d[:])

            # first compute (K-proj of block 0) needs wk + kvt[0]: issue those
            # next, in halves, so the first matmul group's reads unblock early
            wk_t = wp.tile([128, 4096], F32R, tag="w")
            ks0 = stream.tile([128, 4096], F32R, tag="stream")
            for q in range(4):
                s = slice(q * 1024, (q + 1) * 1024)
                nc.sync.dma_start(wk_t[:, s], wk_d[:, s])
                nc.sync.dma_start(ks0[:, s], kvt_d[0, :, s])
            wv_t = wp.tile([128, 4096], F32R, tag="w")
            nc.sync.dma_start(wv_t[:], wv_d[:])

            # bv broadcast across partitions (K=1 matmul with ones column)
            pb = misc_ps(0)
            nc.tensor.matmul(pb[:], ones_t[0:1, :], bv_t[:], start=True, stop=True)
            bvbc = const.tile([128, 512], F32, tag="bvbc")
            nc.vector.tensor_copy(bvbc[:], pb[:])

            kh = khp.tile([128, 8192], F32R)       # KhT: chunk cc at cols cc*2048, kv pos within
            vx = vxp.tile([128, 8320], F32R)       # V_ext: kv chunk jk at cols jk*520, head h at +h*65

            # ---- Phase 1: K-projection + V-projection over 4 kv column blocks
            for kb in range(4):
                if kb == 0:
                    ks = ks0
                else:
                    ks = stream.tile([128, 4096], F32R, tag="stream")
                    nc.sync.dma_start(ks[:], kvt_d[kb])
                for mm in range(4):
                    pp = misc_ps(mm)
                    for c in range(8):
                        nc.tensor.matmul(
                            pp[:],
                            wk_t[:, c * 512 + mm * 128:c * 512 + (mm + 1) * 128],
                            ks[:, c * 512:(c + 1) * 512],
                            start=(c == 0), stop=(c == 7))
                    nc.scalar.activation(
                        kh[:, mm * 2048 + kb * 512:mm * 2048 + (kb + 1) * 512],
                        pp[:], AF.Identity, bias=bk_t[:, mm:mm + 1])
                for js in range(4):
                    jk = 4 * kb + js
                    pp = misc_ps(js)
                    for c in range(8):
                        nc.tensor.matmul(
                            pp[:],
                            ks[:, c * 512 + js * 128:c * 512 + (js + 1) * 128],
                            wv_t[:, c * 512:(c + 1) * 512],
                            start=(c == 0), stop=(c == 7))
                    dst = vx[:, jk * 520:(jk + 1) * 520].rearrange("p (h e) -> p h e", e=65)
                    nc.vector.tensor_tensor(
                        dst[:, :, 0:64],
                        pp[:].rearrange("p (h e) -> p h e", e=64),
                        bvbc[:].rearrange("p (h e) -> p h e", e=64),
                        op=ALU.add)
                    nc.vector.tensor_copy(
                        dst[:, :, 64:65],
                        ones8_t[:].rearrange("p (h e) -> p h e", e=1))

            wq_t = wp.tile([128, 4096], F32R, tag="w")
            nc.sync.dma_start(wq_t[:], wq_d[:])
            wo_t = wp.tile([128, 4096], F32R, tag="w")
            nc.sync.dma_start(wo_t[:], wo_d[:])
            msk_t = const.tile([128, 2048], BF16, tag="msk")
            nc.sync.dma_start(msk_t[:], msk_d[:])
            e2_t = const.tile([128, 128], F32R, tag="e2")
            nc.sync.dma_start(e2_t[:], e2_d[:])

            # ---- Phase 2: software-pipelined.
            # Attention for qb is ACT(exp)-bound; the in-order PE has slack
            # there, so projection matmul groups (Q-proj of qb+1, O-proj of
            # qb-1) are interleaved one unit per kv-pair iteration.
            qh_tiles = {}
            at_tiles = {}

            def qproj_start(qb):
                qs = stream.tile([128, 4096], F32R, tag="stream", name="qs")
                nc.sync.dma_start(qs[:], qt_d[qb])
                qh = qhp.tile([128, 2048], F32R, name="qh")
                qh_tiles[qb] = qh
                return [(qproj_unit, (qb, qs, mm)) for mm in range(4)]

            def qproj_unit(qb, qs, mm):
                pp = misc_ps(mm)
                for c in range(8):
                    nc.tensor.matmul(
                        pp[:],
                        wq_t[:, c * 512 + mm * 128:c * 512 + (mm + 1) * 128],
                        qs[:, c * 512:(c + 1) * 512],
                        start=(c == 0), stop=(c == 7))
                nc.scalar.activation(
                    qh_tiles[qb][:, mm * 512:(mm + 1) * 512],
                    pp[:], AF.Identity, bias=bq_t[:, mm:mm + 1])

            def oproj_unit(qb, u):
                ls, nb = u // 2, u % 2
                at = at_tiles[qb]
                pp = misc_ps(u)
                for c4 in range(4):
                    nc.tensor.matmul(
                        pp[:],
                        at[:, c4 * 512 + ls * 128:c4 * 512 + (ls + 1) * 128],
                        wo_t[:, c4 * 1024 + nb * 512:c4 * 1024 + (nb + 1) * 512],
                        start=(c4 == 0), stop=(c4 == 3))
                ot = osp.tile([128, 512], F32, name="ot")
                nc.vector.tensor_copy(ot[:], pp[:])
                nc.sync.dma_start(
                    out_d[qb * 512 + ls * 128:qb * 512 + (ls + 1) * 128,
                          nb * 512:(nb + 1) * 512], ot[:])

            pending = []
            deferred_norm = [None]

            def drain_one():
                if pending:
                    fn, args = pending.pop(0)
                    fn(*args)

            for fn, args in qproj_start(0):
                fn(*args)   # qb=0 Q-proj runs up front

            for qb in range(4):
                if qb < 3:
                    pending.extend(qproj_start(qb + 1))
                qh = qh_tiles[qb]
                at = atp.tile([128, 2048], F32R, name="at")
                at_tiles[qb] = at
                nkv = 4 * qb + 4
                rec0 = smp.tile([128, 512], F32R, tag="recs0", name="rec0")
                rec1 = smp.tile([128, 512], F32R, tag="recs1", name="rec1")
                qb_slots = 4 * (nkv // 2)
                qb_units = len(pending)
                slot = 0
                drained = 0
                for hp in range(4):
                    # head pair (h0, h1) shares feature chunk hp; h0 occupies PE
                    # row strips 0-63, h1 64-127, so their K=64 scores matmuls
                    # run concurrently in disjoint row groups.
                    h0, h1 = 2 * hp, 2 * hp + 1
                    opx = ps_out.tile([65, 1024], F32, tag="op")
                    op0 = opx[:, 0:512]
                    op1 = opx[:, 512:1024]
                    for jp in range(nkv // 2):
                        jk0, jk1 = 2 * jp, 2 * jp + 1
                        scA = ps_sc.tile([128, 1024], F32, tag="scA")
                        scB = ps_sc.tile([128, 1024], F32, tag="scB")
                        for half, jk in ((0, jk0), (1, jk1)):
                            kcol = hp * 2048 + jk * 128
                            nc.tensor.matmul(
                                scA[:, half * 512:(half + 1) * 512],
                                kh[0:64, kcol:kcol + 128],
                                qh[0:64, hp * 512:(hp + 1) * 512],
                                start=True, stop=True)
                            nc.tensor.matmul(
                                scB[:, half * 512:(half + 1) * 512],
                                kh[64:128, kcol:kcol + 128],
                                qh[64:128, hp * 512:(hp + 1) * 512],
                                start=True, stop=True)
                        if jp == 0 and deferred_norm[0] is not None:
                            # previous pair's normalization drops in here, after
                            # this pair's scores are already queued on the PE
                            deferred_norm[0]()
                            deferred_norm[0] = None
                        t0 = jk0 - 4 * qb
                        if t0 >= 0:
                            mslice = msk_t[:, t0 * 512:(t0 + 2) * 512]
                            nc.vector.tensor_tensor(scA[:], scA[:], mslice, op=ALU.add)
                            nc.vector.tensor_tensor(scB[:], scB[:], mslice, op=ALU.add)
                        slot += 1
                        while pending and drained * qb_slots < slot * qb_units:
                            drain_one()
                            drained += 1
                        prA = prp.tile([128, 1024], F32R, tag="prA")
                        nc.scalar.activation(prA[:], scA[:], AF.Exp)
                        prB = prp.tile([128, 1024], F32R, tag="prB")
                        nc.scalar.activation(prB[:], scB[:], AF.Exp)
                        first, last = (jp == 0), (jp == nkv // 2 - 1)
                        nc.tensor.matmul(
                            op0[:], vx[:, jk0 * 520 + h0 * 65:jk0 * 520 + (h0 + 1) * 65],
                            prA[:, 0:512], start=first, stop=False)
                        nc.tensor.matmul(
                            op0[:], vx[:, jk1 * 520 + h0 * 65:jk1 * 520 + (h0 + 1) * 65],
                            prA[:, 512:1024], start=False, stop=last)
                        nc.tensor.matmul(
                            op1[:], vx[:, jk0 * 520 + h1 * 65:jk0 * 520 + (h1 + 1) * 65],
                            prB[:, 0:512], start=first, stop=False)
                        nc.tensor.matmul(
                            op1[:], vx[:, jk1 * 520 + h1 * 65:jk1 * 520 + (h1 + 1) * 65],
                            prB[:, 512:1024], start=False, stop=last)
                    def emit_norm(hp=hp, at=at, opx=opx, rec0=rec0, rec1=rec1):
                        # stash the unnormalized pair output + its reciprocals
                        # (quadrant rows: DVE partition bases must be 32-aligned);
                        # the division happens in one bulk pass per q block
                        with nc.allow_low_precision(reason="f32r recip for bulk normalize"):
                            for par, rec in ((0, rec0), (1, rec1)):
                                nc.vector.reciprocal(
                                    rec[32 * hp:32 * hp + 1, :],
                                    opx[64:65, par * 512:(par + 1) * 512])
                        for par in (0, 1):
                            nc.vector.tensor_copy(
                                at[64 * par:64 * par + 64, hp * 512:(hp + 1) * 512],
                                opx[0:64, par * 512:(par + 1) * 512])
                    deferred_norm[0] = emit_norm
                if deferred_norm[0] is not None:
                    deferred_norm[0]()
                    deferred_norm[0] = None
                for cc in range(4):
                    for par, rec in ((0, rec0), (1, rec1)):
                        bc = misc_ps(cc * 2 + par, shape=(64, 512))
                        nc.tensor.matmul(
                            bc[:], e2_t[32 * cc:32 * cc + 1, 0:64],
                            rec[32 * cc:32 * cc + 1, :],
                            start=True, stop=True,
                            tile_position=(32 * cc, 0))
                        atsl = at[64 * par:64 * par + 64, cc * 512:(cc + 1) * 512]
                        nc.vector.tensor_tensor(atsl, atsl, bc[:], op=ALU.mult)
                pending.extend([(oproj_unit, (qb, u)) for u in range(8)])

            while pending:
                drain_one()

    nc.compile()
    return nc


def _get_nc():
    if "nc" not in _CACHE:
        _CACHE["nc"] = _build_nc()
    return _CACHE["nc"]


def _prep_w(Wg):
    # W_g.T [1024, 512] -> [128, 4096]: col (c, n) -> c*512 + n; row p = k within chunk
    return np.ascontiguousarray(
        Wg.T.reshape(8, 128, 512).transpose(1, 0, 2).reshape(128, 4096))


def _prep_seqT(x):
    # x [2048, 1024] -> [4, 128, 4096]: [blk][p][c*512 + j] = x[blk*512 + j, c*128 + p]
    return np.ascontiguousarray(
        x.reshape(4, 512, 8, 128).transpose(0, 3, 2, 1).reshape(4, 128, 4096))


def _masks():
    j = np.arange(512)[None, :]
    p = np.arange(128)[:, None]
    cols = []
    for t in range(4):
        cols.append(np.where(j >= 128 * t + p, 0.0, -10000.0))
    m = np.concatenate(cols, axis=1).astype(ml_dtypes.bfloat16)
    return m


def _prep_wo(Wog):
    # Wo[:, g] slice transposed: [512, 1024] -> [128, 4096] col (c, nb, n) -> c*1024 + nb*512 + n
    return np.ascontiguousarray(
        Wog.T.reshape(4, 128, 2, 512).transpose(1, 0, 2, 3).reshape(128, 4096))


def kernel(**inputs):
    from concourse.bass_utils import run_bass_kernel_spmd

    kv = np.asarray(inputs["kv"], np.float32)
    q = np.asarray(inputs["q"], np.float32)
    Wq = np.asarray(inputs["Wq"], np.float32)
    bq = np.asarray(inputs["bq"], np.float32)
    Wk = np.asarray(inputs["Wk"], np.float32)
    bk = np.asarray(inputs["bk"], np.float32)
    Wv = np.asarray(inputs["Wv"], np.float32)
    bv = np.asarray(inputs["bv"], np.float32)
    Wo = np.asarray(inputs["Wo"], np.float32)
    bo = np.asarray(inputs["bo"], np.float32)

    nc = _get_nc()
    msk = _masks()
    ones = np.ones((1, 128), np.float32)
    ones8 = np.ones((128, 8), np.float32)
    e2 = np.zeros((128, 128), np.float32)
    for c in range(4):
        e2[32 * c + 0, 0:64] = 1.0
        e2[32 * c + 1, 64:128] = 1.0

    in_maps = []
    for c in range(NCORES):
        b, g = c // 2, c % 2
        sl = slice(g * 512, (g + 1) * 512)
        in_maps.append({
            "qt": _prep_seqT(q[b]),
            "kvt": _prep_seqT(kv[b]),
            "wq": _prep_w(Wq[sl] * SCALE),
            "wk": _prep_w(Wk[sl]),
            "wv": _prep_w(Wv[sl]),
            "wo": _prep_wo(Wo[:, sl]),
            "bq": np.ascontiguousarray((bq[sl] * SCALE).reshape(4, 128).T),
            "bk": np.ascontiguousarray(bk[sl].reshape(4, 128).T),
            "bv": bv[sl].reshape(1, 512),
            "ones": ones,
            "ones8": ones8,
            "msk": msk,
            "e2": e2,
        })

    res = run_bass_kernel_spmd(nc, in_maps, core_ids=list(range(NCORES)),
                               **_CACHE.get("run_kwargs", {}))
    _CACHE["last_results"] = res
    out = np.empty((B, L, D), np.float32)
    for b in range(B):
        out[b] = res.results[2 * b]["out"] + res.results[2 * b + 1]["out"] + bo[None, :]
    return out



# revision 7
# speedup vs baseline: 1.3127x; 1.3127x over previous
"""Causal cross-attention (B=4, L=2048, D=1024, H=16, hd=64) on 8 trn2 cores.

Sharding: core c -> (batch b = c//2, head-group g = c%2 of 8 heads).
Each core computes QKV projections for its head group, causal-masked
per-head attention, and a partial output projection (its heads' columns
of Wo). Host sums the two partials per batch and adds bo.

Cost-model-aware layout: PE matmul time is (moving columns) x pe_cycle,
independent of K/M, so every matmul streams its SMALL dim:
  scoresT[kv, q] = khT_chunk.T @ qhT      (N = causally-trimmed q cols)
  attn[q, f]    += probsT_slice.T @ V_chk (N = 64 feature cols)
  denom[q]      += probsT_slice.T @ ones  (N = 1)
Normalization is a per-partition DVE scalar multiply (q on partitions),
and the [q, f] -> [f, q] transpose for the O-projection runs on the DMA
XBAR (dma_start_transpose), off the PE entirely. All operands bf16
(1 cycle/row at any N); PSUM accumulation stays f32.
"""

import numpy as np
import ml_dtypes

B, L, D, H, HD = 4, 2048, 1024, 16, 64
NCORES = 8
SCALE = HD ** -0.5

_CACHE = {}


def _build_nc():
    import concourse.mybir as mybir
    import concourse.tile as tile
    from concourse import bacc

    F32 = mybir.dt.float32
    BF16 = mybir.dt.bfloat16
    AF = mybir.ActivationFunctionType
    ALU = mybir.AluOpType

    nc = bacc.Bacc("TRN2", target_bir_lowering=False, debug=False)

    qt_d = nc.declare_dram_parameter("qt", [4, 128, 4096], BF16, isOutput=False)
    kvt_d = nc.declare_dram_parameter("kvt", [4, 128, 4096], BF16, isOutput=False)
    wq_d = nc.declare_dram_parameter("wq", [128, 4096], BF16, isOutput=False)
    wk_d = nc.declare_dram_parameter("wk", [128, 4096], BF16, isOutput=False)
    wv_d = nc.declare_dram_parameter("wv", [128, 4096], BF16, isOutput=False)
    wo_d = nc.declare_dram_parameter("wo", [128, 4096], BF16, isOutput=False)
    bq_d = nc.declare_dram_parameter("bq", [128, 4], F32, isOutput=False)
    bk_d = nc.declare_dram_parameter("bk", [128, 4], F32, isOutput=False)
    bv_d = nc.declare_dram_parameter("bv", [1, 512], BF16, isOutput=False)
    onesr_d = nc.declare_dram_parameter("onesr", [1, 128], BF16, isOutput=False)
    onesc_d = nc.declare_dram_parameter("onesc", [128, 1], BF16, isOutput=False)
    msk_d = nc.declare_dram_parameter("msk", [128, 128], BF16, isOutput=False)
    out_d = nc.declare_dram_parameter("out", [2048, 1024], F32, isOutput=True)

    with tile.TileContext(nc) as tc:
        with (
            tc.tile_pool(name="const", bufs=1) as const,
            tc.tile_pool(name="w", bufs=4) as wp,
            tc.tile_pool(name="ksp", bufs=3) as ksp,
            tc.tile_pool(name="qsp", bufs=2) as qsp,
            tc.tile_pool(name="khp", bufs=1) as khp,
            tc.tile_pool(name="vxp", bufs=1) as vxp,
            tc.tile_pool(name="qhp", bufs=2) as qhp,
            tc.tile_pool(name="prp", bufs=32) as prp,
            tc.tile_pool(name="rcp", bufs=2) as rcp,
            tc.tile_pool(name="atp", bufs=8) as atp,
            tc.tile_pool(name="attp", bufs=8) as attp,
            tc.tile_pool(name="osp", bufs=3) as osp,
            tc.tile_pool(name="ps_sc", bufs=2, space="PSUM") as ps_sc,
            tc.tile_pool(name="ps_av", bufs=2, space="PSUM") as ps_av,
            tc.tile_pool(name="ps_dn", bufs=1, space="PSUM") as ps_dn,
            tc.tile_pool(name="ps_mc", bufs=1, space="PSUM") as ps_mc,
        ):
            # tiny const loads first so the PE's first instruction (the bv
            # broadcast) and the first diag-mask add aren't stuck behind
            # megabytes of weight DMA
            msk_t = const.tile([128, 128], BF16, tag="msk")
            bk_t = const.tile([128, 4], F32, tag="bk")
            bq_t = const.tile([128, 4], F32, tag="bq")
            onesr_t = const.tile([1, 128], BF16, tag="onesr")
            onesc_t = const.tile([128, 1], BF16, tag="onesc")
            bv_t = const.tile([1, 512], BF16, tag="bv")
            for t, d in ((msk_t, msk_d), (bk_t, bk_d), (bq_t, bq_d),
                         (onesr_t, onesr_d), (onesc_t, onesc_d), (bv_t, bv_d)):
                nc.sync.dma_start(t[:], d[:])

            # first compute (kb=0 K/V projection) needs wk + kvt[0]: halves
            # so the first matmul group's operands land early
            wk_t = wp.tile([128, 4096], BF16, tag="w")
            ks_tiles = {}
            ks_tiles[0] = ksp.tile([128, 4096], BF16, tag="ks", name="ks")
            for q in range(4):
                s = slice(q * 1024, (q + 1) * 1024)
                nc.sync.dma_start(wk_t[:, s], wk_d[:, s])
                nc.sync.dma_start(ks_tiles[0][:, s], kvt_d[0, :, s])
            wv_t = wp.tile([128, 4096], BF16, tag="w")
            nc.sync.dma_start(wv_t[:], wv_d[:])
            ks_tiles[1] = ksp.tile([128, 4096], BF16, tag="ks", name="ks")
            nc.sync.dma_start(ks_tiles[1][:], kvt_d[1])
            wq_t = wp.tile([128, 4096], BF16, tag="w")
            nc.sync.dma_start(wq_t[:], wq_d[:])
            qs_tiles = {}
            qs_tiles[0] = qsp.tile([128, 4096], BF16, tag="qs", name="qs")
            nc.sync.dma_start(qs_tiles[0][:], qt_d[0])
            wo_t = wp.tile([128, 4096], BF16, tag="w")
            nc.sync.dma_start(wo_t[:], wo_d[:])

            # bv broadcast across partitions (K=1 matmul with ones column)
            pb = ps_sc.tile([128, 1024], F32, tag="sc")
            nc.tensor.matmul(pb[:, 0:512], onesr_t[0:1, :], bv_t[:],
                             start=True, stop=True)
            bvbc = const.tile([128, 512], F32, tag="bvbc")
            nc.vector.tensor_copy(bvbc[:], pb[:, 0:512])

            kh = khp.tile([128, 8192], BF16)   # [feat(mm slice), mm*2048 + kv]
            vx = vxp.tile([128, 8192], BF16)   # [kv within chunk, jk*512 + h*64 + e]

            def kunit(kb, mm, pool):
                pp = pool.tile([128, 512], F32, tag="av" if pool is ps_av else "mc")
                ks = ks_tiles[kb]
                for c in range(8):
                    nc.tensor.matmul(
                        pp[:],
                        wk_t[:, c * 512 + mm * 128:c * 512 + (mm + 1) * 128],
                        ks[:, c * 512:(c + 1) * 512],
                        start=(c == 0), stop=(c == 7))
                nc.vector.tensor_scalar(
                    kh[:, mm * 2048 + kb * 512:mm * 2048 + (kb + 1) * 512],
                    pp[:], bk_t[:, mm:mm + 1], None, op0=ALU.add)

            def vunit(kb, js, pool):
                pp = pool.tile([128, 512], F32, tag="av" if pool is ps_av else "mc")
                ks = ks_tiles[kb]
                for c in range(8):
                    nc.tensor.matmul(
                        pp[:],
                        ks[:, c * 512 + js * 128:c * 512 + (js + 1) * 128],
                        wv_t[:, c * 512:(c + 1) * 512],
                        start=(c == 0), stop=(c == 7))
                jk = 4 * kb + js
                nc.vector.tensor_tensor(
                    vx[:, jk * 512:(jk + 1) * 512], pp[:], bvbc[:], op=ALU.add)

            # kb=0 K/V projections run inline up front (ps_av is free until
            # the first AV accumulation, well after these drain)
            for mm in range(4):
                kunit(0, mm, ps_av)
            for js in range(4):
                vunit(0, js, ps_av)

            qh_tiles = {}

            def qproj_unit(qb, mm, pool):
                pp = pool.tile([128, 512], F32, tag="sc" if pool is ps_sc else "mc")
                qs = qs_tiles[qb]
                for c in range(8):
                    nc.tensor.matmul(
                        pp[:],
                        wq_t[:, c * 512 + mm * 128:c * 512 + (mm + 1) * 128],
                        qs[:, c * 512:(c + 1) * 512],
                        start=(c == 0), stop=(c == 7))
                nc.vector.tensor_scalar(
                    qh_tiles[qb][:, mm * 512:(mm + 1) * 512],
                    pp[:], bq_t[:, mm:mm + 1], None, op0=ALU.add)

            # qb=0 Q-projection inline (ps_sc ring, free until first scores)
            qh_tiles[0] = qhp.tile([128, 2048], BF16, name="qh")
            for mm in range(4):
                qproj_unit(0, mm, ps_sc)

            at_tiles = {}
            att_tiles = {}
            osb_tiles = {}

            def pref_ks(kb):
                ks_tiles[kb] = ksp.tile([128, 4096], BF16, tag="ks", name="ks")
                nc.sync.dma_start(ks_tiles[kb][:], kvt_d[kb])

            def pref_qs(qb):
                qs_tiles[qb] = qsp.tile([128, 4096], BF16, tag="qs", name="qs")
                nc.sync.dma_start(qs_tiles[qb][:], qt_d[qb])
                qh_tiles[qb] = qhp.tile([128, 2048], BF16, name="qh")

            def oproj_unit(qb, s, nb):
                att = att_tiles[(qb, s)]
                pp = ps_mc.tile([128, 512], F32, tag="mc")
                for i in range(4):
                    nc.tensor.matmul(
                        pp[:],
                        att[:, i * 128:(i + 1) * 128],
                        wo_t[:, i * 1024 + nb * 512:i * 1024 + (nb + 1) * 512],
                        start=(i == 0), stop=(i == 3))
                if nb == 0:
                    osb_tiles[(qb, s)] = osp.tile([128, 1024], F32, name="osb")
                ot = osb_tiles[(qb, s)]
                nc.vector.tensor_copy(ot[:, nb * 512:(nb + 1) * 512], pp[:])
                if nb == 1:
                    nc.sync.dma_start(
                        out_d[qb * 512 + s * 128:qb * 512 + (s + 1) * 128, :],
                        ot[:])

            pending = []

            def drain_one():
                if pending:
                    fn, args = pending.pop(0)
                    fn(*args)

            # ---- attention over 4 q blocks, software-pipelined with the
            # remaining K/V projections, Q projections, and O projections
            for qb in range(4):
                nkv = 4 * qb + 4
                if qb == 0:
                    pending += [(pref_ks, (2,)), (pref_qs, (1,))]
                    pending += [(kunit, (1, mm, ps_mc)) for mm in range(4)]
                    pending += [(vunit, (1, js, ps_mc)) for js in range(4)]
                    pending += [(qproj_unit, (1, mm, ps_mc)) for mm in range(4)]
                elif qb == 1:
                    pending += [(pref_ks, (3,)), (pref_qs, (2,))]
                    pending += [(kunit, (2, mm, ps_mc)) for mm in range(4)]
                    pending += [(vunit, (2, js, ps_mc)) for js in range(4)]
                    pending += [(oproj_unit, (0, s, nb))
                                for s in range(4) for nb in range(2)]
                    pending += [(qproj_unit, (2, mm, ps_mc)) for mm in range(4)]
                elif qb == 2:
                    pending += [(pref_qs, (3,))]
                    pending += [(kunit, (3, mm, ps_mc)) for mm in range(4)]
                    pending += [(vunit, (3, js, ps_mc)) for js in range(4)]
                    pending += [(oproj_unit, (1, s, nb))
                                for s in range(4) for nb in range(2)]
                    pending += [(qproj_unit, (3, mm, ps_mc)) for mm in range(4)]
                else:
                    pending += [(oproj_unit, (2, s, nb))
                                for s in range(4) for nb in range(2)]

                qh = qh_tiles[qb]
                for s in range(4):
                    at_tiles[(qb, s)] = atp.tile([128, 512], BF16, name="at")

                probs = {}       # (hp, jk) -> (tile, N, t)
                qb_slots = 4 * nkv
                qb_units = len(pending)
                slot = 0
                drained = 0

                def emit_scores(hp):
                    nonlocal slot, drained
                    for jk in range(nkv):
                        t = 128 * (jk % 4) if jk // 4 == qb else 0
                        N = 512 - t
                        # h-even scores in PSUM bank 0 (cols 0:N), h-odd in
                        # bank 1 (cols 512:512+N) — a matmul output may not
                        # cross the 512-f32 bank boundary
                        sc = ps_sc.tile([128, 1024], F32, tag="sc")
                        kcol = hp * 2048 + jk * 128
                        nc.tensor.matmul(
                            sc[:, 0:N], kh[0:64, kcol:kcol + 128],
                            qh[0:64, hp * 512 + t:(hp + 1) * 512],
                            start=True, stop=True)
                        nc.tensor.matmul(
                            sc[:, 512:512 + N], kh[64:128, kcol:kcol + 128],
                            qh[64:128, hp * 512 + t:(hp + 1) * 512],
                            start=True, stop=True)
                        if jk // 4 == qb:
                            nc.vector.tensor_tensor(
                                sc[:, 0:128], sc[:, 0:128], msk_t[:], op=ALU.add)
                            nc.vector.tensor_tensor(
                                sc[:, 512:640], sc[:, 512:640], msk_t[:],
                                op=ALU.add)
                        pr = prp.tile([128, 1024], BF16, name="pr")
                        nc.scalar.activation(
                            pr[:].rearrange("p (g n) -> p g n", g=2)[:, :, 0:N],
                            sc[:].rearrange("p (g n) -> p g n", g=2)[:, :, 0:N],
                            AF.Exp)
                        probs[(hp, jk)] = (pr, N, t)
                        slot += 1
                        while pending and drained * qb_slots < slot * qb_units:
                            drain_one()
                            drained += 1

                def emit_av(hp):
                    av = ps_av.tile([128, 512], F32, tag="av")
                    dn = ps_dn.tile([128, 8], F32, tag="dn")
                    for par in range(2):
                        h = 2 * hp + par
                        for s in range(4):
                            last = 4 * qb + s
                            for jk in range(last + 1):
                                pr, N, t = probs[(hp, jk)]
                                off = par * 512 + s * 128 - t
                                nc.tensor.matmul(
                                    av[:, (par * 4 + s) * 64:(par * 4 + s + 1) * 64],
                                    pr[:, off:off + 128],
                                    vx[:, jk * 512 + h * 64:jk * 512 + (h + 1) * 64],
                                    start=(jk == 0), stop=(jk == last))
                            for jk in range(last + 1):
                                pr, N, t = probs[(hp, jk)]
                                off = par * 512 + s * 128 - t
                                nc.tensor.matmul(
                                    dn[:, par * 4 + s:par * 4 + s + 1],
                                    pr[:, off:off + 128],
                                    onesc_t[:],
                                    start=(jk == 0), stop=(jk == last))
                    rec = rcp.tile([128, 8], F32, name="rec")
                    nc.vector.reciprocal(rec[:], dn[:])
                    for par in range(2):
                        h = 2 * hp + par
                        for s in range(4):
                            nc.vector.tensor_scalar(
                                at_tiles[(qb, s)][:, h * 64:(h + 1) * 64],
                                av[:, (par * 4 + s) * 64:(par * 4 + s + 1) * 64],
                                rec[:, par * 4 + s:par * 4 + s + 1],
                                None, op0=ALU.mult)

                prev = None
                for hp in range(4):
                    emit_scores(hp)
                    if prev is not None:
                        emit_av(prev)
                    prev = hp
                emit_av(3)

                for s in range(4):
                    att = attp.tile([128, 512], BF16, name="att")
                    att_tiles[(qb, s)] = att
                    nc.sync.dma_start_transpose(
                        att[:].rearrange("p (i q) -> p i q", q=128),
                        at_tiles[(qb, s)][:])

            pending += [(oproj_unit, (3, s, nb))
                        for s in range(4) for nb in range(2)]
            while pending:
                drain_one()

    nc.compile()
    return nc


def _get_nc():
    if "nc" not in _CACHE:
        _CACHE["nc"] = _build_nc()
    return _CACHE["nc"]


BF = ml_dtypes.bfloat16


def _prep_w(Wg):
    # W_g.T [1024, 512] -> [128, 4096]: col (c, n) -> c*512 + n; row p = k within chunk
    return np.ascontiguousarray(
        Wg.T.reshape(8, 128, 512).transpose(1, 0, 2).reshape(128, 4096)).astype(BF)


def _prep_seqT(x):
    # x [2048, 1024] -> [4, 128, 4096]: [blk][p][c*512 + j] = x[blk*512 + j, c*128 + p]
    return np.ascontiguousarray(
        x.reshape(4, 512, 8, 128).transpose(0, 3, 2, 1).reshape(4, 128, 4096)).astype(BF)


def _prep_wo(Wog):
    # Wo[:, g] slice transposed: [512, 1024] -> [128, 4096] col (i, nb, n) -> i*1024 + nb*512 + n
    return np.ascontiguousarray(
        Wog.T.reshape(4, 128, 2, 512).transpose(1, 0, 2, 3).reshape(128, 4096)).astype(BF)


def _mask():
    j = np.arange(128)[None, :]
    p = np.arange(128)[:, None]
    return np.where(p <= j, 0.0, -10000.0).astype(BF)


def kernel(**inputs):
    from concourse.bass_utils import run_bass_kernel_spmd

    kv = np.asarray(inputs["kv"], np.float32)
    q = np.asarray(inputs["q"], np.float32)
    Wq = np.asarray(inputs["Wq"], np.float32)
    bq = np.asarray(inputs["bq"], np.float32)
    Wk = np.asarray(inputs["Wk"], np.float32)
    bk = np.asarray(inputs["bk"], np.float32)
    Wv = np.asarray(inputs["Wv"], np.float32)
    bv = np.asarray(inputs["bv"], np.float32)
    Wo = np.asarray(inputs["Wo"], np.float32)
    bo = np.asarray(inputs["bo"], np.float32)

    nc = _get_nc()
    msk = _mask()
    onesr = np.ones((1, 128), BF)
    onesc = np.ones((128, 1), BF)

    in_maps = []
    for c in range(NCORES):
        b, g = c // 2, c % 2
        sl = slice(g * 512, (g + 1) * 512)
        in_maps.append({
            "qt": _prep_seqT(q[b]),
            "kvt": _prep_seqT(kv[b]),
            "wq": _prep_w(Wq[sl] * SCALE),
            "wk": _prep_w(Wk[sl]),
            "wv": _prep_w(Wv[sl]),
            "wo": _prep_wo(Wo[:, sl]),
            "bq": np.ascontiguousarray((bq[sl] * SCALE).reshape(4, 128).T),
            "bk": np.ascontiguousarray(bk[sl].reshape(4, 128).T),
            "bv": bv[sl].reshape(1, 512).astype(BF),
            "onesr": onesr,
            "onesc": onesc,
            "msk": msk,
        })

    res = run_bass_kernel_spmd(nc, in_maps, core_ids=list(range(NCORES)),
                               **_CACHE.get("run_kwargs", {}))
    _CACHE["last_results"] = res
    out = np.empty((B, L, D), np.float32)
    for b in range(B):
        out[b] = res.results[2 * b]["out"] + res.results[2 * b + 1]["out"] + bo[None, :]
    return out


# revision 12
# speedup vs baseline: 1.3257x; 1.0099x over previous
"""Causal cross-attention (B=4, L=2048, D=1024, H=16, hd=64) on 8 trn2 cores.

Sharding: core c -> (batch b = c//2, head-group g = c%2 of 8 heads).
Each core computes QKV projections for its head group, causal-masked
per-head attention, and a partial output projection (its heads' columns
of Wo). Host sums the two partials per batch and adds bo.

Cost-model-aware layout: PE matmul time is (moving columns) x pe_cycle,
independent of K/M, so every matmul streams its SMALL dim:
  scoresT[kv, q] = khT_chunk.T @ qhT      (N = causally-trimmed q cols)
  attn[q, f]    += probsT_slice.T @ V_chk (N = 64 feature cols)
  denom[q]      += probsT_slice.T @ ones  (N = 1)
Normalization is a per-partition DVE scalar multiply (q on partitions),
and the [q, f] -> [f, q] transpose for the O-projection runs on the DMA
XBAR (dma_start_transpose), off the PE entirely. All operands bf16
(1 cycle/row at any N); PSUM accumulation stays f32.
"""

import numpy as np
import ml_dtypes

B, L, D, H, HD = 4, 2048, 1024, 16, 64
NCORES = 8
SCALE = HD ** -0.5

_CACHE = {}


def _build_nc():
    import concourse.mybir as mybir
    import concourse.tile as tile
    from concourse import bacc

    F32 = mybir.dt.float32
    BF16 = mybir.dt.bfloat16
    AF = mybir.ActivationFunctionType
    ALU = mybir.AluOpType

    nc = bacc.Bacc("TRN2", target_bir_lowering=False, debug=False)

    qt_d = nc.declare_dram_parameter("qt", [4, 128, 4096], BF16, isOutput=False)
    kvt_d = nc.declare_dram_parameter("kvt", [4, 128, 4096], BF16, isOutput=False)
    wq_d = nc.declare_dram_parameter("wq", [128, 4096], BF16, isOutput=False)
    wk_d = nc.declare_dram_parameter("wk", [128, 4096], BF16, isOutput=False)
    wv_d = nc.declare_dram_parameter("wv", [128, 4096], BF16, isOutput=False)
    wo_d = nc.declare_dram_parameter("wo", [128, 4096], BF16, isOutput=False)
    bq_d = nc.declare_dram_parameter("bq", [128, 4], F32, isOutput=False)
    bk_d = nc.declare_dram_parameter("bk", [128, 4], F32, isOutput=False)
    bv_d = nc.declare_dram_parameter("bv", [1, 512], BF16, isOutput=False)
    onesr_d = nc.declare_dram_parameter("onesr", [1, 128], BF16, isOutput=False)
    onesc_d = nc.declare_dram_parameter("onesc", [128, 1], BF16, isOutput=False)
    msk_d = nc.declare_dram_parameter("msk", [128, 128], BF16, isOutput=False)
    out_d = nc.declare_dram_parameter("out", [2048, 1024], F32, isOutput=True)

    with tile.TileContext(nc) as tc:
        with (
            tc.tile_pool(name="const", bufs=1) as const,
            tc.tile_pool(name="w", bufs=4) as wp,
            tc.tile_pool(name="ksp", bufs=3) as ksp,
            tc.tile_pool(name="qsp", bufs=2) as qsp,
            tc.tile_pool(name="khp", bufs=1) as khp,
            tc.tile_pool(name="vxp", bufs=1) as vxp,
            tc.tile_pool(name="qhp", bufs=2) as qhp,
            tc.tile_pool(name="prp", bufs=32) as prp,
            tc.tile_pool(name="rcp", bufs=2) as rcp,
            tc.tile_pool(name="atp", bufs=8) as atp,
            tc.tile_pool(name="attp", bufs=8) as attp,
            tc.tile_pool(name="osp", bufs=3) as osp,
            tc.tile_pool(name="ps_sc", bufs=2, space="PSUM") as ps_sc,
            tc.tile_pool(name="ps_av", bufs=2, space="PSUM") as ps_av,
            tc.tile_pool(name="ps_dn", bufs=1, space="PSUM") as ps_dn,
            tc.tile_pool(name="ps_mc", bufs=1, space="PSUM") as ps_mc,
        ):
            # tiny const loads first so the PE's first instruction (the bv
            # broadcast) and the first diag-mask add aren't stuck behind
            # megabytes of weight DMA
            msk_t = const.tile([128, 128], BF16, tag="msk")
            bk_t = const.tile([128, 4], F32, tag="bk")
            bq_t = const.tile([128, 4], F32, tag="bq")
            onesr_t = const.tile([1, 128], BF16, tag="onesr")
            onesc_t = const.tile([128, 1], BF16, tag="onesc")
            bv_t = const.tile([1, 512], BF16, tag="bv")
            for t, d in ((onesr_t, onesr_d), (bv_t, bv_d), (msk_t, msk_d),
                         (bk_t, bk_d), (bq_t, bq_d), (onesc_t, onesc_d)):
                nc.sync.dma_start(t[:], d[:])

            # first compute (kb=0 K/V projection) needs wk + kvt[0]: halves
            # so the first matmul group's operands land early
            wk_t = wp.tile([128, 4096], BF16, tag="w")
            ks_tiles = {}
            ks_tiles[0] = ksp.tile([128, 4096], BF16, tag="ks", name="ks")
            for q in range(4):
                s = slice(q * 1024, (q + 1) * 1024)
                nc.sync.dma_start(wk_t[:, s], wk_d[:, s])
                nc.sync.dma_start(ks_tiles[0][:, s], kvt_d[0, :, s])
            wv_t = wp.tile([128, 4096], BF16, tag="w")
            nc.sync.dma_start(wv_t[:], wv_d[:])
            ks_tiles[1] = ksp.tile([128, 4096], BF16, tag="ks", name="ks")
            nc.sync.dma_start(ks_tiles[1][:], kvt_d[1])
            wq_t = wp.tile([128, 4096], BF16, tag="w")
            nc.sync.dma_start(wq_t[:], wq_d[:])
            qs_tiles = {}
            qs_tiles[0] = qsp.tile([128, 4096], BF16, tag="qs", name="qs")
            nc.sync.dma_start(qs_tiles[0][:], qt_d[0])
            wo_t = wp.tile([128, 4096], BF16, tag="w")
            nc.sync.dma_start(wo_t[:], wo_d[:])

            # bv broadcast across partitions (K=1 matmul with ones column)
            pb = ps_sc.tile([128, 1024], F32, tag="sc")
            nc.tensor.matmul(pb[:, 0:512], onesr_t[0:1, :], bv_t[:],
                             start=True, stop=True)
            bvbc = const.tile([128, 512], F32, tag="bvbc")
            nc.vector.tensor_copy(bvbc[:], pb[:, 0:512])

            kh = khp.tile([128, 8192], BF16)   # [feat(mm slice), mm*2048 + kv]
            vx = vxp.tile([128, 8192], BF16)   # [kv within chunk, jk*512 + h*64 + e]

            def kunit(kb, mm, pool):
                pp = pool.tile([128, 512], F32, tag="av" if pool is ps_av else "mc")
                ks = ks_tiles[kb]
                for c in range(8):
                    nc.tensor.matmul(
                        pp[:],
                        wk_t[:, c * 512 + mm * 128:c * 512 + (mm + 1) * 128],
                        ks[:, c * 512:(c + 1) * 512],
                        start=(c == 0), stop=(c == 7))
                nc.vector.tensor_scalar(
                    kh[:, mm * 2048 + kb * 512:mm * 2048 + (kb + 1) * 512],
                    pp[:], bk_t[:, mm:mm + 1], None, op0=ALU.add)

            def vunit(kb, js, pool):
                pp = pool.tile([128, 512], F32, tag="av" if pool is ps_av else "mc")
                ks = ks_tiles[kb]
                for c in range(8):
                    nc.tensor.matmul(
                        pp[:],
                        ks[:, c * 512 + js * 128:c * 512 + (js + 1) * 128],
                        wv_t[:, c * 512:(c + 1) * 512],
                        start=(c == 0), stop=(c == 7))
                jk = 4 * kb + js
                nc.vector.tensor_tensor(
                    vx[:, jk * 512:(jk + 1) * 512], pp[:], bvbc[:], op=ALU.add)

            # kb=0 K/V projections run inline up front (ps_av is free until
            # the first AV accumulation, well after these drain)
            for mm in range(4):
                kunit(0, mm, ps_av)
            for js in range(4):
                vunit(0, js, ps_av)

            qh_tiles = {}

            def qproj_unit(qb, mm, pool):
                pp = pool.tile([128, 512], F32, tag="sc" if pool is ps_sc else "mc")
                qs = qs_tiles[qb]
                for c in range(8):
                    nc.tensor.matmul(
                        pp[:],
                        wq_t[:, c * 512 + mm * 128:c * 512 + (mm + 1) * 128],
                        qs[:, c * 512:(c + 1) * 512],
                        start=(c == 0), stop=(c == 7))
                nc.vector.tensor_scalar(
                    qh_tiles[qb][:, mm * 512:(mm + 1) * 512],
                    pp[:], bq_t[:, mm:mm + 1], None, op0=ALU.add)

            # qb=0 Q-projection inline (ps_sc ring, free until first scores)
            qh_tiles[0] = qhp.tile([128, 2048], BF16, name="qh")
            for mm in range(4):
                qproj_unit(0, mm, ps_sc)

            at_tiles = {}
            att_tiles = {}
            osb_tiles = {}

            def pref_ks(kb):
                ks_tiles[kb] = ksp.tile([128, 4096], BF16, tag="ks", name="ks")
                nc.sync.dma_start(ks_tiles[kb][:], kvt_d[kb])

            def pref_qs(qb):
                qs_tiles[qb] = qsp.tile([128, 4096], BF16, tag="qs", name="qs")
                nc.sync.dma_start(qs_tiles[qb][:], qt_d[qb])
                qh_tiles[qb] = qhp.tile([128, 2048], BF16, name="qh")

            def oproj_unit(qb, s, nb, pool=None):
                att = att_tiles[(qb, s)]
                pool = pool or ps_mc
                pp = pool.tile([128, 512], F32,
                               tag="av" if pool is ps_av else "mc")
                for i in range(4):
                    nc.tensor.matmul(
                        pp[:],
                        att[:, i * 128:(i + 1) * 128],
                        wo_t[:, i * 1024 + nb * 512:i * 1024 + (nb + 1) * 512],
                        start=(i == 0), stop=(i == 3))
                if nb == 0:
                    osb_tiles[(qb, s)] = osp.tile([128, 1024], F32, name="osb")
                ot = osb_tiles[(qb, s)]
                nc.vector.tensor_copy(ot[:, nb * 512:(nb + 1) * 512], pp[:])
                if nb == 1:
                    nc.sync.dma_start(
                        out_d[qb * 512 + s * 128:qb * 512 + (s + 1) * 128, :],
                        ot[:])

            pending = []

            def drain_one():
                if pending:
                    fn, args = pending.pop(0)
                    fn(*args)

            # ---- attention over 4 q blocks, software-pipelined with the
            # remaining K/V projections, Q projections, and O projections
            for qb in range(4):
                nkv = 4 * qb + 4
                if qb == 0:
                    pending += [(pref_ks, (2,)), (pref_qs, (1,))]
                    pending += [(kunit, (1, mm, ps_mc)) for mm in range(4)]
                    pending += [(vunit, (1, js, ps_mc)) for js in range(4)]
                    pending += [(qproj_unit, (1, mm, ps_mc)) for mm in range(4)]
                elif qb == 1:
                    pending += [(pref_ks, (3,)), (pref_qs, (2,))]
                    pending += [(kunit, (2, mm, ps_mc)) for mm in range(4)]
                    pending += [(vunit, (2, js, ps_mc)) for js in range(4)]
                    pending += [(oproj_unit, (0, s, nb))
                                for s in range(4) for nb in range(2)]
                    pending += [(qproj_unit, (2, mm, ps_mc)) for mm in range(4)]
                elif qb == 2:
                    pending += [(pref_qs, (3,))]
                    pending += [(kunit, (3, mm, ps_mc)) for mm in range(4)]
                    pending += [(vunit, (3, js, ps_mc)) for js in range(4)]
                    pending += [(oproj_unit, (1, s, nb))
                                for s in range(4) for nb in range(2)]
                    pending += [(qproj_unit, (3, mm, ps_mc)) for mm in range(4)]
                else:
                    pending += [(oproj_unit, (2, s, nb))
                                for s in range(4) for nb in range(2)]

                qh = qh_tiles[qb]
                for s in range(4):
                    at_tiles[(qb, s)] = atp.tile([128, 512], BF16, name="at")

                probs = {}       # (hp, jk) -> (tile, N, t)
                qb_slots = 4 * nkv
                qb_units = len(pending)
                slot = 0
                drained = 0

                def emit_scores(hp):
                    nonlocal slot, drained
                    for jk in range(nkv):
                        t = 128 * (jk % 4) if jk // 4 == qb else 0
                        N = 512 - t
                        # h-even scores in PSUM bank 0 (cols 0:N), h-odd in
                        # bank 1 (cols 512:512+N) — a matmul output may not
                        # cross the 512-f32 bank boundary
                        sc = ps_sc.tile([128, 1024], F32, tag="sc")
                        kcol = hp * 2048 + jk * 128
                        nc.tensor.matmul(
                            sc[:, 0:N], kh[0:64, kcol:kcol + 128],
                            qh[0:64, hp * 512 + t:(hp + 1) * 512],
                            start=True, stop=True)
                        nc.tensor.matmul(
                            sc[:, 512:512 + N], kh[64:128, kcol:kcol + 128],
                            qh[64:128, hp * 512 + t:(hp + 1) * 512],
                            start=True, stop=True)
                        if jk // 4 == qb:
                            nc.vector.tensor_tensor(
                                sc[:, 0:128], sc[:, 0:128], msk_t[:], op=ALU.add)
                            nc.vector.tensor_tensor(
                                sc[:, 512:640], sc[:, 512:640], msk_t[:],
                                op=ALU.add)
                        pr = prp.tile([128, 1024], BF16, name="pr")
                        nc.scalar.activation(
                            pr[:].rearrange("p (g n) -> p g n", g=2)[:, :, 0:N],
                            sc[:].rearrange("p (g n) -> p g n", g=2)[:, :, 0:N],
                            AF.Exp)
                        probs[(hp, jk)] = (pr, N, t)
                        slot += 1
                        while pending and drained * qb_slots < slot * qb_units:
                            drain_one()
                            drained += 1

                def emit_av(hp):
                    # region-outer: PSUM allows only one open accumulation
                    # group per bank, so each (par, s) region must run
                    # start..stop before the next begins (av and dn are in
                    # different banks, so their groups may interleave)
                    av = ps_av.tile([128, 512], F32, tag="av")
                    dn = ps_dn.tile([128, 8], F32, tag="dn")
                    for par in range(2):
                        h = 2 * hp + par
                        for s in range(4):
                            last = 4 * qb + s
                            for jk in range(last + 1):
                                pr, N, t = probs[(hp, jk)]
                                off = par * 512 + s * 128 - t
                                nc.tensor.matmul(
                                    av[:, (par * 4 + s) * 64:(par * 4 + s + 1) * 64],
                                    pr[:, off:off + 128],
                                    vx[:, jk * 512 + h * 64:jk * 512 + (h + 1) * 64],
                                    start=(jk == 0), stop=(jk == last))
                                nc.tensor.matmul(
                                    dn[:, par * 4 + s:par * 4 + s + 1],
                                    pr[:, off:off + 128],
                                    onesc_t[:],
                                    start=(jk == 0), stop=(jk == last))
                    rec = rcp.tile([128, 8], F32, name="rec")
                    nc.vector.reciprocal(rec[:], dn[:])
                    for par in range(2):
                        h = 2 * hp + par
                        for s in range(4):
                            nc.vector.tensor_scalar(
                                at_tiles[(qb, s)][:, h * 64:(h + 1) * 64],
                                av[:, (par * 4 + s) * 64:(par * 4 + s + 1) * 64],
                                rec[:, par * 4 + s:par * 4 + s + 1],
                                None, op0=ALU.mult)

                prev = None
                for hp in range(4):
                    emit_scores(hp)
                    if prev is not None:
                        emit_av(prev)
                    prev = hp
                emit_av(3)

                for s in range(4):
                    att = attp.tile([128, 512], BF16, name="att")
                    att_tiles[(qb, s)] = att
                    nc.sync.dma_start_transpose(
                        att[:].rearrange("p (i q) -> p i q", q=128),
                        at_tiles[(qb, s)][:])

            # tail: ps_av is free after qb=3's last normalize, so the final
            # O-projection batch double-buffers there instead of serializing
            # on the 1-deep ps_mc
            pending += [(oproj_unit, (3, s, nb, ps_av))
                        for s in range(4) for nb in range(2)]
            while pending:
                drain_one()

    nc.compile()
    return nc


def _get_nc():
    if "nc" not in _CACHE:
        _CACHE["nc"] = _build_nc()
    return _CACHE["nc"]


BF = ml_dtypes.bfloat16


def _prep_w(Wg):
    # W_g.T [1024, 512] -> [128, 4096]: col (c, n) -> c*512 + n; row p = k within chunk
    return np.ascontiguousarray(
        Wg.T.reshape(8, 128, 512).transpose(1, 0, 2).reshape(128, 4096)).astype(BF)


def _prep_seqT(x):
    # x [2048, 1024] -> [4, 128, 4096]: [blk][p][c*512 + j] = x[blk*512 + j, c*128 + p]
    return np.ascontiguousarray(
        x.reshape(4, 512, 8, 128).transpose(0, 3, 2, 1).reshape(4, 128, 4096)).astype(BF)


def _prep_wo(Wog):
    # Wo[:, g] slice transposed: [512, 1024] -> [128, 4096] col (i, nb, n) -> i*1024 + nb*512 + n
    return np.ascontiguousarray(
        Wog.T.reshape(4, 128, 2, 512).transpose(1, 0, 2, 3).reshape(128, 4096)).astype(BF)


def _mask():
    j = np.arange(128)[None, :]
    p = np.arange(128)[:, None]
    return np.where(p <= j, 0.0, -10000.0).astype(BF)


def kernel(**inputs):
    from concourse.bass_utils import run_bass_kernel_spmd

    kv = np.asarray(inputs["kv"], np.float32)
    q = np.asarray(inputs["q"], np.float32)
    Wq = np.asarray(inputs["Wq"], np.float32)
    bq = np.asarray(inputs["bq"], np.float32)
    Wk = np.asarray(inputs["Wk"], np.float32)
    bk = np.asarray(inputs["bk"], np.float32)
    Wv = np.asarray(inputs["Wv"], np.float32)
    bv = np.asarray(inputs["bv"], np.float32)
    Wo = np.asarray(inputs["Wo"], np.float32)
    bo = np.asarray(inputs["bo"], np.float32)

    nc = _get_nc()
    msk = _mask()
    onesr = np.ones((1, 128), BF)
    onesc = np.ones((128, 1), BF)

    in_maps = []
    for c in range(NCORES):
        b, g = c // 2, c % 2
        sl = slice(g * 512, (g + 1) * 512)
        in_maps.append({
            "qt": _prep_seqT(q[b]),
            "kvt": _prep_seqT(kv[b]),
            "wq": _prep_w(Wq[sl] * SCALE),
            "wk": _prep_w(Wk[sl]),
            "wv": _prep_w(Wv[sl]),
            "wo": _prep_wo(Wo[:, sl]),
            "bq": np.ascontiguousarray((bq[sl] * SCALE).reshape(4, 128).T),
            "bk": np.ascontiguousarray(bk[sl].reshape(4, 128).T),
            "bv": bv[sl].reshape(1, 512).astype(BF),
            "onesr": onesr,
            "onesc": onesc,
            "msk": msk,
        })

    res = run_bass_kernel_spmd(nc, in_maps, core_ids=list(range(NCORES)),
                               **_CACHE.get("run_kwargs", {}))
    _CACHE["last_results"] = res
    out = np.empty((B, L, D), np.float32)
    for b in range(B):
        out[b] = res.results[2 * b]["out"] + res.results[2 * b + 1]["out"] + bo[None, :]
    return out


# revision 15
# speedup vs baseline: 1.3266x; 1.0007x over previous
"""Causal cross-attention (B=4, L=2048, D=1024, H=16, hd=64) on 8 trn2 cores.

Sharding: core c -> (batch b = c//2, head-group g = c%2 of 8 heads).
Each core computes QKV projections for its head group, causal-masked
per-head attention, and a partial output projection (its heads' columns
of Wo). Host sums the two partials per batch and adds bo.

Cost-model-aware layout: PE matmul time is (moving columns) x pe_cycle,
independent of K/M, so every matmul streams its SMALL dim:
  scoresT[kv, q] = khT_chunk.T @ qhT      (N = causally-trimmed q cols)
  attn[q, f]    += probsT_slice.T @ V_chk (N = 64 feature cols)
  denom[q]      += probsT_slice.T @ ones  (N = 1)
Normalization is a per-partition DVE scalar multiply (q on partitions),
and the [q, f] -> [f, q] transpose for the O-projection runs on the DMA
XBAR (dma_start_transpose), off the PE entirely. All operands bf16
(1 cycle/row at any N); PSUM accumulation stays f32.
"""

import numpy as np
import ml_dtypes

B, L, D, H, HD = 4, 2048, 1024, 16, 64
NCORES = 8
SCALE = HD ** -0.5

_CACHE = {}


def _build_nc():
    import concourse.mybir as mybir
    import concourse.tile as tile
    from concourse import bacc

    F32 = mybir.dt.float32
    BF16 = mybir.dt.bfloat16
    AF = mybir.ActivationFunctionType
    ALU = mybir.AluOpType

    nc = bacc.Bacc("TRN2", target_bir_lowering=False, debug=False)

    qt_d = nc.declare_dram_parameter("qt", [4, 128, 4096], BF16, isOutput=False)
    kvt_d = nc.declare_dram_parameter("kvt", [4, 128, 4096], BF16, isOutput=False)
    wq_d = nc.declare_dram_parameter("wq", [128, 4096], BF16, isOutput=False)
    wk_d = nc.declare_dram_parameter("wk", [128, 4096], BF16, isOutput=False)
    wv_d = nc.declare_dram_parameter("wv", [128, 4096], BF16, isOutput=False)
    wo_d = nc.declare_dram_parameter("wo", [128, 4096], BF16, isOutput=False)
    bq_d = nc.declare_dram_parameter("bq", [128, 4], F32, isOutput=False)
    bk_d = nc.declare_dram_parameter("bk", [128, 4], F32, isOutput=False)
    bv_d = nc.declare_dram_parameter("bv", [1, 512], BF16, isOutput=False)
    onesr_d = nc.declare_dram_parameter("onesr", [1, 128], BF16, isOutput=False)
    onesc_d = nc.declare_dram_parameter("onesc", [128, 1], BF16, isOutput=False)
    msk_d = nc.declare_dram_parameter("msk", [128, 128], BF16, isOutput=False)
    out_d = nc.declare_dram_parameter("out", [2048, 1024], F32, isOutput=True)

    with tile.TileContext(nc) as tc:
        with (
            tc.tile_pool(name="const", bufs=1) as const,
            tc.tile_pool(name="w", bufs=4) as wp,
            tc.tile_pool(name="ksp", bufs=2) as ksp,
            tc.tile_pool(name="qsp", bufs=2) as qsp,
            tc.tile_pool(name="khp", bufs=1) as khp,
            tc.tile_pool(name="vxp", bufs=1) as vxp,
            tc.tile_pool(name="qhp", bufs=2) as qhp,
            tc.tile_pool(name="prp", bufs=36) as prp,
            tc.tile_pool(name="rcp", bufs=2) as rcp,
            tc.tile_pool(name="atp", bufs=8) as atp,
            tc.tile_pool(name="attp", bufs=8) as attp,
            tc.tile_pool(name="osp", bufs=3) as osp,
            tc.tile_pool(name="ps_sc", bufs=2, space="PSUM") as ps_sc,
            tc.tile_pool(name="ps_av", bufs=2, space="PSUM") as ps_av,
            tc.tile_pool(name="ps_dn", bufs=1, space="PSUM") as ps_dn,
            tc.tile_pool(name="ps_mc", bufs=1, space="PSUM") as ps_mc,
        ):
            # tiny const loads first so the PE's first instruction (the bv
            # broadcast) and the first diag-mask add aren't stuck behind
            # megabytes of weight DMA
            msk_t = const.tile([128, 128], BF16, tag="msk")
            bk_t = const.tile([128, 4], F32, tag="bk")
            bq_t = const.tile([128, 4], F32, tag="bq")
            onesr_t = const.tile([1, 128], BF16, tag="onesr")
            onesc_t = const.tile([128, 1], BF16, tag="onesc")
            bv_t = const.tile([1, 512], BF16, tag="bv")
            for t, d in ((onesr_t, onesr_d), (bv_t, bv_d), (msk_t, msk_d),
                         (bk_t, bk_d), (bq_t, bq_d), (onesc_t, onesc_d)):
                nc.sync.dma_start(t[:], d[:])

            # first compute (kb=0 K/V projection) needs wk + kvt[0]: halves
            # so the first matmul group's operands land early
            wk_t = wp.tile([128, 4096], BF16, tag="w")
            ks_tiles = {}
            ks_tiles[0] = ksp.tile([128, 4096], BF16, tag="ks", name="ks")
            for q in range(4):
                s = slice(q * 1024, (q + 1) * 1024)
                nc.sync.dma_start(wk_t[:, s], wk_d[:, s])
                nc.sync.dma_start(ks_tiles[0][:, s], kvt_d[0, :, s])
            wv_t = wp.tile([128, 4096], BF16, tag="w")
            nc.sync.dma_start(wv_t[:], wv_d[:])
            ks_tiles[1] = ksp.tile([128, 4096], BF16, tag="ks", name="ks")
            nc.sync.dma_start(ks_tiles[1][:], kvt_d[1])
            wq_t = wp.tile([128, 4096], BF16, tag="w")
            nc.sync.dma_start(wq_t[:], wq_d[:])
            qs_tiles = {}
            qs_tiles[0] = qsp.tile([128, 4096], BF16, tag="qs", name="qs")
            nc.sync.dma_start(qs_tiles[0][:], qt_d[0])
            wo_t = wp.tile([128, 4096], BF16, tag="w")
            nc.sync.dma_start(wo_t[:], wo_d[:])

            # bv broadcast across partitions (K=1 matmul with ones column)
            pb = ps_sc.tile([128, 1024], F32, tag="sc")
            nc.tensor.matmul(pb[:, 0:512], onesr_t[0:1, :], bv_t[:],
                             start=True, stop=True)
            bvbc = const.tile([128, 512], F32, tag="bvbc")
            nc.vector.tensor_copy(bvbc[:], pb[:, 0:512])

            kh = khp.tile([128, 8192], BF16)   # [feat(mm slice), mm*2048 + kv]
            vx = vxp.tile([128, 8192], BF16)   # [kv within chunk, jk*512 + h*64 + e]

            def kunit(kb, mm, pool):
                pp = pool.tile([128, 512], F32, tag="av" if pool is ps_av else "mc")
                ks = ks_tiles[kb]
                for c in range(8):
                    nc.tensor.matmul(
                        pp[:],
                        wk_t[:, c * 512 + mm * 128:c * 512 + (mm + 1) * 128],
                        ks[:, c * 512:(c + 1) * 512],
                        start=(c == 0), stop=(c == 7))
                nc.vector.tensor_scalar(
                    kh[:, mm * 2048 + kb * 512:mm * 2048 + (kb + 1) * 512],
                    pp[:], bk_t[:, mm:mm + 1], None, op0=ALU.add)

            def vunit(kb, js, pool):
                pp = pool.tile([128, 512], F32, tag="av" if pool is ps_av else "mc")
                ks = ks_tiles[kb]
                for c in range(8):
                    nc.tensor.matmul(
                        pp[:],
                        ks[:, c * 512 + js * 128:c * 512 + (js + 1) * 128],
                        wv_t[:, c * 512:(c + 1) * 512],
                        start=(c == 0), stop=(c == 7))
                jk = 4 * kb + js
                nc.vector.tensor_tensor(
                    vx[:, jk * 512:(jk + 1) * 512], pp[:], bvbc[:], op=ALU.add)

            # kb=0 K/V projections run inline up front (ps_av is free until
            # the first AV accumulation, well after these drain)
            for mm in range(4):
                kunit(0, mm, ps_av)
            for js in range(4):
                vunit(0, js, ps_av)

            qh_tiles = {}

            def qproj_unit(qb, mm, pool):
                pp = pool.tile([128, 512], F32, tag="sc" if pool is ps_sc else "mc")
                qs = qs_tiles[qb]
                for c in range(8):
                    nc.tensor.matmul(
                        pp[:],
                        wq_t[:, c * 512 + mm * 128:c * 512 + (mm + 1) * 128],
                        qs[:, c * 512:(c + 1) * 512],
                        start=(c == 0), stop=(c == 7))
                nc.vector.tensor_scalar(
                    qh_tiles[qb][:, mm * 512:(mm + 1) * 512],
                    pp[:], bq_t[:, mm:mm + 1], None, op0=ALU.add)

            # qb=0 Q-projection inline (ps_sc ring, free until first scores)
            qh_tiles[0] = qhp.tile([128, 2048], BF16, name="qh")
            for mm in range(4):
                qproj_unit(0, mm, ps_sc)

            at_tiles = {}
            att_tiles = {}
            osb_tiles = {}

            def pref_ks(kb):
                ks_tiles[kb] = ksp.tile([128, 4096], BF16, tag="ks", name="ks")
                nc.sync.dma_start(ks_tiles[kb][:], kvt_d[kb])

            def pref_qs(qb):
                qs_tiles[qb] = qsp.tile([128, 4096], BF16, tag="qs", name="qs")
                nc.sync.dma_start(qs_tiles[qb][:], qt_d[qb])
                qh_tiles[qb] = qhp.tile([128, 2048], BF16, name="qh")

            def oproj_unit(qb, s, nb, pool=None):
                att = att_tiles[(qb, s)]
                pool = pool or ps_mc
                pp = pool.tile([128, 512], F32,
                               tag="av" if pool is ps_av else "mc")
                for i in range(4):
                    nc.tensor.matmul(
                        pp[:],
                        att[:, i * 128:(i + 1) * 128],
                        wo_t[:, i * 1024 + nb * 512:i * 1024 + (nb + 1) * 512],
                        start=(i == 0), stop=(i == 3))
                if nb == 0:
                    osb_tiles[(qb, s)] = osp.tile([128, 1024], F32, name="osb")
                ot = osb_tiles[(qb, s)]
                nc.vector.tensor_copy(ot[:, nb * 512:(nb + 1) * 512], pp[:])
                if nb == 1:
                    nc.sync.dma_start(
                        out_d[qb * 512 + s * 128:qb * 512 + (s + 1) * 128, :],
                        ot[:])

            pending = []

            def drain_one():
                if pending:
                    fn, args = pending.pop(0)
                    fn(*args)

            # ---- attention over 4 q blocks, software-pipelined with the
            # remaining K/V projections, Q projections, and O projections
            for qb in range(4):
                nkv = 4 * qb + 4
                if qb == 0:
                    pending += [(pref_ks, (2,)), (pref_qs, (1,))]
                    pending += [(kunit, (1, mm, ps_mc)) for mm in range(4)]
                    pending += [(vunit, (1, js, ps_mc)) for js in range(4)]
                    pending += [(qproj_unit, (1, mm, ps_mc)) for mm in range(4)]
                elif qb == 1:
                    pending += [(pref_ks, (3,)), (pref_qs, (2,))]
                    pending += [(kunit, (2, mm, ps_mc)) for mm in range(4)]
                    pending += [(vunit, (2, js, ps_mc)) for js in range(4)]
                    pending += [(oproj_unit, (0, s, nb))
                                for s in range(4) for nb in range(2)]
                    pending += [(qproj_unit, (2, mm, ps_mc)) for mm in range(4)]
                elif qb == 2:
                    pending += [(pref_qs, (3,))]
                    pending += [(kunit, (3, mm, ps_mc)) for mm in range(4)]
                    pending += [(vunit, (3, js, ps_mc)) for js in range(4)]
                    pending += [(oproj_unit, (1, s, nb))
                                for s in range(4) for nb in range(2)]
                    pending += [(qproj_unit, (3, mm, ps_mc)) for mm in range(4)]
                else:
                    pending += [(oproj_unit, (2, s, nb))
                                for s in range(4) for nb in range(2)]

                qh = qh_tiles[qb]
                for s in range(4):
                    at_tiles[(qb, s)] = atp.tile([128, 512], BF16, name="at")

                probs = {}       # (hp, jk) -> (tile, N, t)
                qb_slots = 4 * nkv
                qb_units = len(pending)
                slot = 0
                drained = 0

                def emit_scores(hp):
                    nonlocal slot, drained
                    for jk in range(nkv):
                        t = 128 * (jk % 4) if jk // 4 == qb else 0
                        N = 512 - t
                        # h-even scores in PSUM bank 0 (cols 0:N), h-odd in
                        # bank 1 (cols 512:512+N) — a matmul output may not
                        # cross the 512-f32 bank boundary
                        sc = ps_sc.tile([128, 1024], F32, tag="sc")
                        kcol = hp * 2048 + jk * 128
                        nc.tensor.matmul(
                            sc[:, 0:N], kh[0:64, kcol:kcol + 128],
                            qh[0:64, hp * 512 + t:(hp + 1) * 512],
                            start=True, stop=True)
                        nc.tensor.matmul(
                            sc[:, 512:512 + N], kh[64:128, kcol:kcol + 128],
                            qh[64:128, hp * 512 + t:(hp + 1) * 512],
                            start=True, stop=True)
                        if jk // 4 == qb:
                            nc.vector.tensor_tensor(
                                sc[:, 0:128], sc[:, 0:128], msk_t[:], op=ALU.add)
                            nc.vector.tensor_tensor(
                                sc[:, 512:640], sc[:, 512:640], msk_t[:],
                                op=ALU.add)
                        pr = prp.tile([128, 1024], BF16, name="pr")
                        nc.scalar.activation(
                            pr[:].rearrange("p (g n) -> p g n", g=2)[:, :, 0:N],
                            sc[:].rearrange("p (g n) -> p g n", g=2)[:, :, 0:N],
                            AF.Exp)
                        probs[(hp, jk)] = (pr, N, t)
                        slot += 1
                        # qb=3 has only the oproj(2) batch left to drain;
                        # hold it for the second half, where the exp stream
                        # is the deepest behind and the PE needs filler
                        eff_slot = slot if qb < 3 else max(0, 2 * slot - qb_slots)
                        while pending and drained * qb_slots < eff_slot * qb_units:
                            drain_one()
                            drained += 1

                def emit_av(hp):
                    # region-outer: PSUM allows only one open accumulation
                    # group per bank, so each (par, s) region must run
                    # start..stop before the next begins (av and dn are in
                    # different banks, so their groups may interleave)
                    av = ps_av.tile([128, 512], F32, tag="av")
                    dn = ps_dn.tile([128, 8], F32, tag="dn")
                    for par in range(2):
                        h = 2 * hp + par
                        for s in range(4):
                            last = 4 * qb + s
                            for jk in range(last + 1):
                                pr, N, t = probs[(hp, jk)]
                                off = par * 512 + s * 128 - t
                                nc.tensor.matmul(
                                    av[:, (par * 4 + s) * 64:(par * 4 + s + 1) * 64],
                                    pr[:, off:off + 128],
                                    vx[:, jk * 512 + h * 64:jk * 512 + (h + 1) * 64],
                                    start=(jk == 0), stop=(jk == last))
                                nc.tensor.matmul(
                                    dn[:, par * 4 + s:par * 4 + s + 1],
                                    pr[:, off:off + 128],
                                    onesc_t[:],
                                    start=(jk == 0), stop=(jk == last))
                    rec = rcp.tile([128, 8], F32, name="rec")
                    nc.vector.reciprocal(rec[:], dn[:])
                    for par in range(2):
                        h = 2 * hp + par
                        for s in range(4):
                            nc.vector.tensor_scalar(
                                at_tiles[(qb, s)][:, h * 64:(h + 1) * 64],
                                av[:, (par * 4 + s) * 64:(par * 4 + s + 1) * 64],
                                rec[:, par * 4 + s:par * 4 + s + 1],
                                None, op0=ALU.mult)

                prev = None
                for hp in range(4):
                    emit_scores(hp)
                    if prev is not None:
                        emit_av(prev)
                    prev = hp
                emit_av(3)

                for s in range(4):
                    att = attp.tile([128, 512], BF16, name="att")
                    att_tiles[(qb, s)] = att
                    nc.sync.dma_start_transpose(
                        att[:].rearrange("p (i q) -> p i q", q=128),
                        at_tiles[(qb, s)][:])

            # tail: ps_av is free after qb=3's last normalize, so the final
            # O-projection batch double-buffers there instead of serializing
            # on the 1-deep ps_mc
            pending += [(oproj_unit, (3, s, nb, ps_av))
                        for s in range(4) for nb in range(2)]
            while pending:
                drain_one()

    nc.compile()
    return nc


def _get_nc():
    if "nc" not in _CACHE:
        _CACHE["nc"] = _build_nc()
    return _CACHE["nc"]


BF = ml_dtypes.bfloat16


def _prep_w(Wg):
    # W_g.T [1024, 512] -> [128, 4096]: col (c, n) -> c*512 + n; row p = k within chunk
    return np.ascontiguousarray(
        Wg.T.reshape(8, 128, 512).transpose(1, 0, 2).reshape(128, 4096)).astype(BF)


def _prep_seqT(x):
    # x [2048, 1024] -> [4, 128, 4096]: [blk][p][c*512 + j] = x[blk*512 + j, c*128 + p]
    return np.ascontiguousarray(
        x.reshape(4, 512, 8, 128).transpose(0, 3, 2, 1).reshape(4, 128, 4096)).astype(BF)


def _prep_wo(Wog):
    # Wo[:, g] slice transposed: [512, 1024] -> [128, 4096] col (i, nb, n) -> i*1024 + nb*512 + n
    return np.ascontiguousarray(
        Wog.T.reshape(4, 128, 2, 512).transpose(1, 0, 2, 3).reshape(128, 4096)).astype(BF)


def _mask():
    j = np.arange(128)[None, :]
    p = np.arange(128)[:, None]
    return np.where(p <= j, 0.0, -10000.0).astype(BF)


def kernel(**inputs):
    from concourse.bass_utils import run_bass_kernel_spmd

    kv = np.asarray(inputs["kv"], np.float32)
    q = np.asarray(inputs["q"], np.float32)
    Wq = np.asarray(inputs["Wq"], np.float32)
    bq = np.asarray(inputs["bq"], np.float32)
    Wk = np.asarray(inputs["Wk"], np.float32)
    bk = np.asarray(inputs["bk"], np.float32)
    Wv = np.asarray(inputs["Wv"], np.float32)
    bv = np.asarray(inputs["bv"], np.float32)
    Wo = np.asarray(inputs["Wo"], np.float32)
    bo = np.asarray(inputs["bo"], np.float32)

    nc = _get_nc()
    msk = _mask()
    onesr = np.ones((1, 128), BF)
    onesc = np.ones((128, 1), BF)

    in_maps = []
    for c in range(NCORES):
        b, g = c // 2, c % 2
        sl = slice(g * 512, (g + 1) * 512)
        in_maps.append({
            "qt": _prep_seqT(q[b]),
            "kvt": _prep_seqT(kv[b]),
            "wq": _prep_w(Wq[sl] * SCALE),
            "wk": _prep_w(Wk[sl]),
            "wv": _prep_w(Wv[sl]),
            "wo": _prep_wo(Wo[:, sl]),
            "bq": np.ascontiguousarray((bq[sl] * SCALE).reshape(4, 128).T),
            "bk": np.ascontiguousarray(bk[sl].reshape(4, 128).T),
            "bv": bv[sl].reshape(1, 512).astype(BF),
            "onesr": onesr,
            "onesc": onesc,
            "msk": msk,
        })

    res = run_bass_kernel_spmd(nc, in_maps, core_ids=list(range(NCORES)),
                               **_CACHE.get("run_kwargs", {}))
    _CACHE["last_results"] = res
    out = np.empty((B, L, D), np.float32)
    for b in range(B):
        out[b] = res.results[2 * b]["out"] + res.results[2 * b + 1]["out"] + bo[None, :]
    return out
